# revision 3
# baseline (speedup 1.0000x reference)
"""Trainium2 Bass kernel for nn_EquivariantProductBasisBlock (MACE product-basis block).

Self-contained: host-side sharding/preprocessing + Bass/Tile device kernel on 8 cores.

Math (validated vs reference): per node n, channel c, species s, x = x[n,c,:] in R^9:
    f[z] = sum_i x_i * F[z,i],   F[zi] = sum_m V[m] * W[c,m,zi]
where V = 56 slots (45 deg-2 monomials | 9 x | 1 one | 1 pad) and W is the
species/channel coefficient table folded from (u*, w*) on the host.
Then gate = f0 @ gate_kernel[s] + gate_bias[s]; f0*=gate[:C]; f1*=gate[C:];
out = [f0 @ lin0, f1 @ lin1] / sqrt(C).

Device layout (node-major): nodes on SBUF partitions (128/tile), species-sorted so
every tile is species-pure (tile t = species t, 10 tiles/core). The heavy per-channel
contraction F = V @ W runs on the PE as one matmul per (channel, tile) with
K=56 monomial slots: lhsT = host-precomputed V^T [56, 128n] (even/odd channel halves
stored separately so all APs are partition-offset-0), rhs = W[s,c] [56, 36].
F accumulates in PSUM in groups of 4 channel-pairs; ACT casts to bf16; DVE does the
final x-contraction (mul + reduce over i) in 2x mode. Gate + equivariant linear are
small per-tile PE matmuls via PE-transposes of the node-major activations.

Species overflow beyond 1024 nodes/species is computed on the host in numpy and
merged (tiny for the target distribution).
"""

import numpy as np

N_CORES = 8
C, D, S = 128, 9, 10
NM = 45           # deg-2 monomials
NROW = 56         # per-channel slots: 45 y | 9 x | 1 one | 1 pad
NPAIR = C // 2    # 64
TILE_N = 128
TPC = S           # tiles per core (one per species)
NODES_PER_CORE = TPC * TILE_N          # 1280
CAP_PER_SPECIES = N_CORES * TILE_N     # 1024 device-handled nodes per species
KH = NROW         # matmul contraction depth per channel half
FV = NPAIR * TILE_N   # 8192: V^T free size per half
FW = NPAIR * 36       # 2304: W free size per half
GRP = 4           # channel-pairs per PSUM group (4*72 floats = 1152B < bank)
NGRP = NPAIR // GRP   # 16

# monomials ordered by diagonal offset o=k-j then j: slot(o,j) = OSTART[o]+j.
OSTART = [0] * D
for o in range(1, D):
    OSTART[o] = OSTART[o - 1] + (D - (o - 1))
MONO_JK = [(j, j + o) for o in range(D) for j in range(D - o)]


# ----------------------------------------------------------------------------
# host math
# ----------------------------------------------------------------------------

def _build_xr(node_feats):
    n = node_feats.shape[0]
    x = np.empty((n, C, D), np.float32)
    x[:, :, 0] = node_feats[:, :C]
    x[:, :, 1:4] = node_feats[:, C:4 * C].reshape(n, C, 3)
    x[:, :, 4:9] = node_feats[:, 4 * C:].reshape(n, C, 5)
    return x


def _build_coeff_tables(i):
    def c3h(u3, w3):
        c3 = np.einsum('zijkp,spc->sczijk', u3, w3[:, :, :], optimize=True)
        out = np.zeros(c3.shape[:4] + (NM,), np.float64)
        for m, (j, k) in enumerate(MONO_JK):
            out[..., m] = c3[..., j, k] if j == k else c3[..., j, k] + c3[..., k, j]
        return out

    def c2(u2, w2):
        return np.einsum('zijp,spc->sczij', u2, w2, optimize=True)

    def c1(u1, w1):
        return np.einsum('zip,spc->sczi', u1, w1, optimize=True)

    h0 = c3h(i['u3_0e'], i['w3_0e']); h1 = c3h(i['u3_1o'], i['w3_1o'])
    q0 = c2(i['u2_0e'], i['w2_0e']);  q1 = c2(i['u2_1o'], i['w2_1o'])
    l0 = c1(i['u1_0e'], i['w1_0e']);  l1 = c1(i['u1_1o'], i['w1_1o'])

    W = np.zeros((S, C, NROW, 36), np.float64)
    W[:, :, 0:45, 0:9] = np.moveaxis(h0[:, :, 0], -1, -2)
    W[:, :, 45:54, 0:9] = np.moveaxis(q0[:, :, 0], -1, -2)
    W[:, :, 54, 0:9] = l0[:, :, 0]
    for z in range(3):
        sl = slice(9 + z * 9, 18 + z * 9)
        W[:, :, 0:45, sl] = np.moveaxis(h1[:, :, z], -1, -2)
        W[:, :, 45:54, sl] = np.moveaxis(q1[:, :, z], -1, -2)
        W[:, :, 54, sl] = l1[:, :, z]
    return W.astype(np.float32)   # [S, C, 56, 36]


def _build_v56(xr):
    """xr [n, C, 9] -> V slots [n, C, 56]."""
    n = xr.shape[0]
    V = np.empty((n, C, NROW), np.float32)
    for m, (j, k) in enumerate(MONO_JK):
        V[:, :, m] = xr[:, :, j] * xr[:, :, k]
    V[:, :, 45:54] = xr
    V[:, :, 54] = 1.0
    V[:, :, 55] = 0.0
    return V


def _numpy_forward(inputs, idx):
    """Reference-equivalent host computation for node subset idx (overflow path)."""
    i = {k: np.asarray(v) for k, v in inputs.items()}
    nf = i['node_feats'][idx]; sp = i['node_species'][idx]
    xr = _build_xr(nf)
    W = _build_coeff_tables(i)
    n = nf.shape[0]
    V = _build_v56(xr)
    F = np.einsum('ncm,ncmz->ncz', V, W[sp], optimize=True)
    f = np.einsum('nczi,nci->ncz', F.reshape(n, C, 4, D), xr, optimize=True)
    f0, f1 = f[:, :, 0], f[:, :, 1:4]
    gate = np.einsum('nc,nck->nk', f0, i['gate_kernel'][sp], optimize=True) + i['gate_bias'][sp]
    f0g = f0 * gate[:, :C]
    f1g = f1 * gate[:, C:, None]
    inv = 1.0 / np.sqrt(np.float32(C))
    o0 = np.einsum('nc,ck->nk', f0g, i['lin_w_0e'], optimize=True) * inv
    o1 = np.einsum('ncd,ck->nkd', f1g, i['lin_w_1o'], optimize=True) * inv
    return np.concatenate([o0.reshape(n, C), o1.reshape(n, C * 3)], axis=1).astype(np.float32)


def _bf16(x):
    import ml_dtypes
    return np.asarray(x, np.float32).astype(ml_dtypes.bfloat16)


def host_prepare(inputs):
    """Returns (per_core_inmaps, device_rows [N_CORES,1280] global node ids (-1 pad),
    overflow_idx)."""
    i = {k: np.asarray(v) for k, v in inputs.items()}
    sp = i['node_species']

    order = np.argsort(sp, kind='stable')
    sorted_sp = sp[order]
    device_rows = np.full((N_CORES, NODES_PER_CORE), -1, np.int64)
    overflow = []
    for s in range(S):
        ids = order[sorted_sp == s]
        dev = ids[:CAP_PER_SPECIES]
        overflow.append(ids[CAP_PER_SPECIES:])
        for k in range(N_CORES):
            chunk = dev[k * TILE_N:(k + 1) * TILE_N]
            device_rows[k, s * TILE_N: s * TILE_N + len(chunk)] = chunk
    overflow_idx = np.concatenate(overflow) if overflow else np.zeros(0, np.int64)

    xr = _build_xr(i['node_feats'])                       # [N, C, 9]
    W = _build_coeff_tables(i)                            # [S, C, 56, 36]

    # W split into even/odd channel halves: [S*56, 64*36], rows (t*56 + m)
    Wh = W.reshape(S, NPAIR, 2, NROW, 36).transpose(0, 2, 3, 1, 4)  # [S,2,56,64,36]
    wce = _bf16(np.ascontiguousarray(Wh[:, 0]).reshape(S * KH, FW))
    wco = _bf16(np.ascontiguousarray(Wh[:, 1]).reshape(S * KH, FW))

    gk = np.zeros((C, S * 2 * C), np.float32)             # rows c, col s*256 + k2
    for s in range(S):
        gk[:, s * 256:(s + 1) * 256] = i['gate_kernel'][s]
    gk_bf = _bf16(gk)

    bias_bc = np.zeros((C, S * 2 * C), np.float32)        # partition-replicated bias
    for s in range(S):
        bias_bc[:, s * 256:(s + 1) * 256] = i['gate_bias'][s][None, :]

    inv = 1.0 / np.sqrt(np.float32(C))
    lin_bf = _bf16(np.concatenate(
        [i['lin_w_0e'] * inv, i['lin_w_1o'] * inv], axis=1))  # [128, 256]

    in_maps = []
    for k in range(N_CORES):
        rows = device_rows[k]
        valid = rows >= 0
        xr_core = np.zeros((NODES_PER_CORE, C, D), np.float32)
        xr_core[valid] = xr[rows[valid]]
        v56 = _build_v56(xr_core.reshape(-1, C, D))       # [1280, 128, 56]
        # -> [tiles, half, m, pair, node]
        vt = v56.reshape(TPC, TILE_N, NPAIR, 2, NROW).transpose(0, 3, 4, 2, 1)
        in_maps.append({
            'vte': _bf16(np.ascontiguousarray(vt[:, 0]).reshape(TPC * KH, FV)),
            'vto': _bf16(np.ascontiguousarray(vt[:, 1]).reshape(TPC * KH, FV)),
            'xn': _bf16(xr_core.reshape(NODES_PER_CORE, C * D)),
            'wce': wce,
            'wco': wco,
            'gk': gk_bf,
            'bias': bias_bc,
            'lin': lin_bf,
        })
    return in_maps, device_rows, overflow_idx


# ----------------------------------------------------------------------------
# device kernel
# ----------------------------------------------------------------------------

def build_device(repeat=1, stages=5):
    import concourse.bacc as bacc
    import concourse.mybir as mybir
    from concourse.tile import TileContext
    from concourse.masks import make_identity

    f32, bf16 = mybir.dt.float32, mybir.dt.bfloat16
    AL = mybir.AluOpType

    nc = bacc.Bacc("TRN2", target_bir_lowering=False, debug=False,
                   num_devices=N_CORES)

    vte_d = nc.dram_tensor('vte', [TPC * KH, FV], bf16, kind='ExternalInput').ap()
    vto_d = nc.dram_tensor('vto', [TPC * KH, FV], bf16, kind='ExternalInput').ap()
    xn_d = nc.dram_tensor('xn', [NODES_PER_CORE, C * D], bf16, kind='ExternalInput').ap()
    wce_d = nc.dram_tensor('wce', [S * KH, FW], bf16, kind='ExternalInput').ap()
    wco_d = nc.dram_tensor('wco', [S * KH, FW], bf16, kind='ExternalInput').ap()
    gk_d = nc.dram_tensor('gk', [C, S * 2 * C], bf16, kind='ExternalInput').ap()
    bias_d = nc.dram_tensor('bias', [C, S * 2 * C], f32, kind='ExternalInput').ap()
    lin_d = nc.dram_tensor('lin', [C, 2 * C], bf16, kind='ExternalInput').ap()
    out_d = nc.dram_tensor('out', [NODES_PER_CORE, 4 * C], f32,
                           kind='ExternalOutput').ap()

    with TileContext(nc) as tc:
        with (
            tc.tile_pool(name='const', bufs=1) as constp,
            tc.tile_pool(name='vte', bufs=2) as vtep,
            tc.tile_pool(name='vto', bufs=2) as vtop,
            tc.tile_pool(name='wc', bufs=2) as wcp,
            tc.tile_pool(name='xn', bufs=2) as xnp,
            tc.tile_pool(name='fsb', bufs=4) as fsbp,
            tc.tile_pool(name='tmp', bufs=4) as tmpp,
            tc.tile_pool(name='facc', bufs=2) as faccp,
            tc.tile_pool(name='fg', bufs=2) as fgp,
            tc.tile_pool(name='xt', bufs=4) as xtp,
            tc.tile_pool(name='gate', bufs=2) as gatep,
            tc.tile_pool(name='ot', bufs=2) as otp,
            tc.tile_pool(name='ps_f', bufs=4, space='PSUM') as ps_f,
            tc.tile_pool(name='ps_t', bufs=2, space='PSUM') as ps_t,
            tc.tile_pool(name='ps_g', bufs=1, space='PSUM') as ps_g,
            tc.tile_pool(name='ps_o', bufs=1, space='PSUM') as ps_o,
        ):
            gk_s = constp.tile([C, S * 2 * C], bf16)
            nc.sync.dma_start(out=gk_s[:], in_=gk_d[:])
            bias_s = constp.tile([C, S * 2 * C], f32)
            nc.sync.dma_start(out=bias_s[:], in_=bias_d[:])
            lin_s = constp.tile([C, 2 * C], bf16)
            nc.sync.dma_start(out=lin_s[:], in_=lin_d[:])
            ident = constp.tile([TILE_N, TILE_N], bf16)
            make_identity(nc, ident[:])

            for rep in range(repeat):
                for t in range(TPC):
                    s = t  # species == tile index
                    vte_t = vtep.tile([KH, FV], bf16)
                    nc.sync.dma_start(out=vte_t[:],
                                      in_=vte_d[t * KH:(t + 1) * KH, :])
                    vto_t = vtop.tile([KH, FV], bf16)
                    nc.sync.dma_start(out=vto_t[:],
                                      in_=vto_d[t * KH:(t + 1) * KH, :])
                    wce_t = wcp.tile([KH, FW], bf16, tag='wce')
                    nc.sync.dma_start(out=wce_t[:],
                                      in_=wce_d[s * KH:(s + 1) * KH, :])
                    wco_t = wcp.tile([KH, FW], bf16, tag='wco')
                    nc.sync.dma_start(out=wco_t[:],
                                      in_=wco_d[s * KH:(s + 1) * KH, :])
                    xn_t = xnp.tile([TILE_N, C * D], bf16)
                    nc.sync.dma_start(out=xn_t[:],
                                      in_=xn_d[t * TILE_N:(t + 1) * TILE_N, :])

                    facc = faccp.tile([TILE_N, 4 * C], bf16)

                    for g in range(NGRP):
                        fps = ps_f.tile([TILE_N, GRP * 72], f32, tag='F')
                        for q in range(GRP):
                            p = GRP * g + q
                            nc.tensor.matmul(
                                fps[:, q * 72:q * 72 + 36],
                                vte_t[:, p * TILE_N:(p + 1) * TILE_N],
                                wce_t[:, p * 36:(p + 1) * 36],
                                start=True, stop=True)
                            nc.tensor.matmul(
                                fps[:, q * 72 + 36:q * 72 + 72],
                                vto_t[:, p * TILE_N:(p + 1) * TILE_N],
                                wco_t[:, p * 36:(p + 1) * 36],
                                start=True, stop=True)
                        fsb = fsbp.tile([TILE_N, GRP * 72], bf16)
                        nc.scalar.copy(fsb[:], fps[:])
                        tmp = tmpp.tile([TILE_N, GRP * 72], bf16)
                        xv = (xn_t[:, (2 * GRP * g) * D:(2 * GRP * g + 2 * GRP) * D]
                              .rearrange('p (c i) -> p c i', c=2 * GRP)
                              .unsqueeze(2).broadcast_to([TILE_N, 2 * GRP, 4, D]))
                        nc.vector.tensor_mul(
                            tmp[:, :].rearrange('p (c z i) -> p c z i',
                                                c=2 * GRP, z=4),
                            fsb[:, :].rearrange('p (c z i) -> p c z i',
                                                c=2 * GRP, z=4),
                            xv)
                        with nc.allow_low_precision(reason='9-elem reduce in bf16'):
                            nc.vector.tensor_reduce(
                                facc[:, g * 8 * GRP:(g + 1) * 8 * GRP],
                                tmp[:, :].rearrange('p (cz i) -> p cz i', i=D),
                                axis=mybir.AxisListType.X, op=AL.add)

                    # ---- gate: gate[n, 2C] = f0^T.T @ gk[s] + bias[s] ----
                    facc_z = facc[:, :].rearrange('p (c z) -> p z c', z=4)
                    tps = ps_t.tile([TILE_N, TILE_N], bf16, tag='tp')
                    nc.tensor.transpose(tps[:], facc_z[:, 0, :], ident[:])
                    f0T = xtp.tile([TILE_N, TILE_N], bf16, tag='f0T')
                    nc.scalar.copy(f0T[:], tps[:])
                    gps = ps_g.tile([TILE_N, 2 * C], f32, tag='g')
                    nc.tensor.matmul(gps[:], f0T[:],
                                     gk_s[:, s * 256:(s + 1) * 256],
                                     start=True, stop=True)
                    gate = gatep.tile([TILE_N, 2 * C], bf16)
                    nc.vector.tensor_add(gate[:], gps[:],
                                         bias_s[:, s * 256:(s + 1) * 256])

                    # ---- apply gates (node-major) ----
                    fg = fgp.tile([TILE_N, 4 * C], bf16)
                    fg_z = fg[:, :].rearrange('p (c z) -> p z c', z=4)
                    nc.vector.tensor_mul(fg_z[:, 0, :], facc_z[:, 0, :],
                                         gate[:, 0:C])
                    fg_c = fg[:, :].rearrange('p (c z) -> p c z', z=4)
                    facc_c = facc[:, :].rearrange('p (c z) -> p c z', z=4)
                    gv = (gate[:, C:2 * C].unsqueeze(2)
                          .broadcast_to([TILE_N, C, 3]))
                    nc.vector.tensor_mul(fg_c[:, :, 1:4], facc_c[:, :, 1:4], gv)

                    # ---- equivariant linear ----
                    ops_ = ps_o.tile([TILE_N, 4 * C], f32, tag='o')
                    for z in range(4):
                        tpz = ps_t.tile([TILE_N, TILE_N], bf16, tag='tp')
                        nc.tensor.transpose(tpz[:], fg_z[:, z, :], ident[:])
                        fgzT = xtp.tile([TILE_N, TILE_N], bf16, tag='fgzT')
                        nc.scalar.copy(fgzT[:], tpz[:])
                        rhs = lin_s[:, 0:C] if z == 0 else lin_s[:, C:2 * C]
                        nc.tensor.matmul(ops_[:, z * C:(z + 1) * C],
                                         fgzT[:], rhs, start=True, stop=True)

                    ot = otp.tile([TILE_N, 4 * C], f32)
                    nc.scalar.copy(ot[:, 0:C], ops_[:, 0:C])
                    nc.vector.tensor_copy(
                        ot[:, C:].rearrange('p (k z) -> p z k', z=3),
                        ops_[:, C:].rearrange('p (z k) -> p z k', z=3))
                    nc.sync.dma_start(
                        out=out_d[t * TILE_N:(t + 1) * TILE_N, :], in_=ot[:])

    nc.compile()
    return nc


_NC_CACHE = {}


def _get_device(repeat=1, stages=5):
    key = (repeat, stages)
    if key not in _NC_CACHE:
        _NC_CACHE[key] = build_device(repeat, stages)
    return _NC_CACHE[key]


def kernel(**inputs):
    from concourse.bass_utils import run_bass_kernel_spmd

    in_maps, device_rows, overflow_idx = host_prepare(inputs)
    nc = _get_device(1)
    res = run_bass_kernel_spmd(nc, in_maps, list(range(N_CORES)))

    ntot = np.asarray(inputs['node_species']).shape[0]
    out = np.zeros((ntot, 4 * C), np.float32)
    for k in range(N_CORES):
        rows = device_rows[k]
        valid = rows >= 0
        o = res.results[k]['out']            # [1280, 512] node-major
        out[rows[valid]] = o[valid]
    if len(overflow_idx):
        out[overflow_idx] = _numpy_forward(inputs, overflow_idx)
    return out


# revision 5
# speedup vs baseline: 8.2669x; 8.2669x over previous
"""Trainium2 Bass kernel for nn_EquivariantProductBasisBlock (MACE product-basis block).

Self-contained: host-side sharding/preprocessing + Bass/Tile device kernel on 8 cores.

Math (validated vs reference): per node n, channel c, species s, x = x[n,c,:] in R^9:
    out[z] = sum_i x_i * F[z,i],   F = C1 + C2 @ x + C3h @ y,   y = {x_j x_k}_{j<=k}
with C* the species/channel coefficient tables folded from (u*, w*) on the host.
Then gate = f0 @ gate_kernel[s] + gate_bias[s]; f0*=gate[:C]; f1*=gate[C:];
out = [f0 @ lin0, f1 @ lin1] / sqrt(C).

Device layout: channels on SBUF partitions, nodes species-sorted so every tile is
species-pure (tile t = species t, 10 tiles/core). The runtime bills ~40-50us per
instruction regardless of engine or size, so the program minimizes instruction
count: one bulk input DMA, monomial build batched over tile pairs (3D APs),
the per-tile V@W contraction in 8 broadcast-mul + 8 reduce ops (SBUF-bound
granularity), fused bias+gating via scalar_tensor_tensor, and per-pair output DMA.

Species overflow beyond 1024 nodes/species is computed on the host in numpy.
"""

import numpy as np

N_CORES = 8
C, D, S = 128, 9, 10
NM = 45           # deg-2 monomials
NROW = 56         # host slots: 45 y | 9 x | 1 one | 1 pad (overflow path)
NM55 = 55         # device slots per channel: 45 y | 9 x | 1 one
TILE_N = 128
TPC = S           # tiles per core (one per species)
NODES_PER_CORE = TPC * TILE_N          # 1280
CAP_PER_SPECIES = N_CORES * TILE_N     # 1024 device-handled nodes per species

# monomials ordered by diagonal offset o=k-j then j: slot(o,j) = OSTART[o]+j.
# Each V-build op is then pure step-1 (out/in0/in1 all contiguous runs).
OSTART = [0] * D
for o in range(1, D):
    OSTART[o] = OSTART[o - 1] + (D - (o - 1))
MONO_JK = [(j, j + o) for o in range(D) for j in range(D - o)]


# ----------------------------------------------------------------------------
# host math
# ----------------------------------------------------------------------------

def _build_xr(node_feats):
    n = node_feats.shape[0]
    x = np.empty((n, C, D), np.float32)
    x[:, :, 0] = node_feats[:, :C]
    x[:, :, 1:4] = node_feats[:, C:4 * C].reshape(n, C, 3)
    x[:, :, 4:9] = node_feats[:, 4 * C:].reshape(n, C, 5)
    return x


def _build_coeff_tables(i):
    def c3h(u3, w3):
        c3 = np.einsum('zijkp,spc->sczijk', u3, w3[:, :, :], optimize=True)
        out = np.zeros(c3.shape[:4] + (NM,), np.float64)
        for m, (j, k) in enumerate(MONO_JK):
            out[..., m] = c3[..., j, k] if j == k else c3[..., j, k] + c3[..., k, j]
        return out

    def c2(u2, w2):
        return np.einsum('zijp,spc->sczij', u2, w2, optimize=True)

    def c1(u1, w1):
        return np.einsum('zip,spc->sczi', u1, w1, optimize=True)

    h0 = c3h(i['u3_0e'], i['w3_0e']); h1 = c3h(i['u3_1o'], i['w3_1o'])
    q0 = c2(i['u2_0e'], i['w2_0e']);  q1 = c2(i['u2_1o'], i['w2_1o'])
    l0 = c1(i['u1_0e'], i['w1_0e']);  l1 = c1(i['u1_1o'], i['w1_1o'])

    W = np.zeros((S, C, NROW, 36), np.float64)
    W[:, :, 0:45, 0:9] = np.moveaxis(h0[:, :, 0], -1, -2)
    W[:, :, 45:54, 0:9] = np.moveaxis(q0[:, :, 0], -1, -2)
    W[:, :, 54, 0:9] = l0[:, :, 0]
    for z in range(3):
        sl = slice(9 + z * 9, 18 + z * 9)
        W[:, :, 0:45, sl] = np.moveaxis(h1[:, :, z], -1, -2)
        W[:, :, 45:54, sl] = np.moveaxis(q1[:, :, z], -1, -2)
        W[:, :, 54, sl] = l1[:, :, z]
    return W.astype(np.float32)   # [S, C, 56, 36]


def _numpy_forward(inputs, idx):
    """Reference-equivalent host computation for node subset idx (overflow path)."""
    i = {k: np.asarray(v) for k, v in inputs.items()}
    nf = i['node_feats'][idx]; sp = i['node_species'][idx]
    xr = _build_xr(nf)
    W = _build_coeff_tables(i)
    n = nf.shape[0]
    V = np.empty((n, C, NROW), np.float32)
    for m, (j, k) in enumerate(MONO_JK):
        V[:, :, m] = xr[:, :, j] * xr[:, :, k]
    V[:, :, 45:54] = xr
    V[:, :, 54] = 1.0
    V[:, :, 55] = 0.0
    F = np.einsum('ncm,ncmz->ncz', V, W[sp], optimize=True)
    f = np.einsum('nczi,nci->ncz', F.reshape(n, C, 4, D), xr, optimize=True)
    f0, f1 = f[:, :, 0], f[:, :, 1:4]
    gate = np.einsum('nc,nck->nk', f0, i['gate_kernel'][sp], optimize=True) + i['gate_bias'][sp]
    f0g = f0 * gate[:, :C]
    f1g = f1 * gate[:, C:, None]
    inv = 1.0 / np.sqrt(np.float32(C))
    o0 = np.einsum('nc,ck->nk', f0g, i['lin_w_0e'], optimize=True) * inv
    o1 = np.einsum('ncd,ck->nkd', f1g, i['lin_w_1o'], optimize=True) * inv
    return np.concatenate([o0.reshape(n, C), o1.reshape(n, C * 3)], axis=1).astype(np.float32)


def _bf16(x):
    import ml_dtypes
    return np.asarray(x, np.float32).astype(ml_dtypes.bfloat16)


def host_prepare(inputs):
    """Returns (per_core_inmaps, device_rows [N_CORES,1280] global node ids (-1 pad),
    overflow_idx)."""
    i = {k: np.asarray(v) for k, v in inputs.items()}
    sp = i['node_species']

    order = np.argsort(sp, kind='stable')
    sorted_sp = sp[order]
    device_rows = np.full((N_CORES, NODES_PER_CORE), -1, np.int64)
    overflow = []
    for s in range(S):
        ids = order[sorted_sp == s]
        dev = ids[:CAP_PER_SPECIES]
        overflow.append(ids[CAP_PER_SPECIES:])
        for k in range(N_CORES):
            chunk = dev[k * TILE_N:(k + 1) * TILE_N]
            device_rows[k, s * TILE_N: s * TILE_N + len(chunk)] = chunk
    overflow_idx = np.concatenate(overflow) if overflow else np.zeros(0, np.int64)

    xr = _build_xr(i['node_feats'])                       # [N, C, 9]
    W = _build_coeff_tables(i)                            # [S, C, 56, 36]
    # channel-major coefficient table: wc[c, (s, m55, zi36)]
    wc_bf = _bf16(np.ascontiguousarray(
        W[:, :, :55, :].transpose(1, 0, 2, 3).reshape(C, S * 55 * 36)))

    gk = np.zeros((C, S * 2 * C), np.float32)             # rows c, col s*256 + j
    for s in range(S):
        gk[:, s * 256:(s + 1) * 256] = i['gate_kernel'][s]

    bias = np.zeros((C, S * 2), np.float32)               # rows k2%128, col s*2 + half
    for s in range(S):
        bias[:, 2 * s] = i['gate_bias'][s, :C]
        bias[:, 2 * s + 1] = i['gate_bias'][s, C:]

    inv = 1.0 / np.sqrt(np.float32(C))
    lin = np.concatenate([i['lin_w_0e'] * inv, i['lin_w_1o'] * inv], axis=1)  # [128, 256]

    gk_bf = _bf16(gk); lin_bf = _bf16(lin)

    in_maps = []
    for k in range(N_CORES):
        rows = device_rows[k]
        xr_core = np.zeros((NODES_PER_CORE, C * D), np.float32)
        valid = rows >= 0
        xr_core[valid] = xr[rows[valid]].reshape(-1, C * D)
        # channel-major x: xt[c, (tile, i, node)]
        xt = xr_core.reshape(TPC, TILE_N, C, D).transpose(2, 0, 3, 1)
        in_maps.append({
            'xt': _bf16(np.ascontiguousarray(xt.reshape(C, TPC * D * TILE_N))),
            'wc': wc_bf,
            'gk': gk_bf,
            'bias': bias,
            'lin': lin_bf,
        })
    return in_maps, device_rows, overflow_idx


# ----------------------------------------------------------------------------
# device kernel
# ----------------------------------------------------------------------------

def build_device(repeat=1, stages=5):
    import concourse.bacc as bacc
    import concourse.mybir as mybir
    from concourse.tile import TileContext

    f32, bf16 = mybir.dt.float32, mybir.dt.bfloat16
    AL = mybir.AluOpType

    nc = bacc.Bacc("TRN2", target_bir_lowering=False, debug=False,
                   num_devices=N_CORES)

    xt_d = nc.dram_tensor('xt', [C, TPC * D * TILE_N], bf16, kind='ExternalInput').ap()
    wc_d = nc.dram_tensor('wc', [C, S * NM55 * 36], bf16, kind='ExternalInput').ap()
    gk_d = nc.dram_tensor('gk', [C, S * 2 * C], bf16, kind='ExternalInput').ap()
    bias_d = nc.dram_tensor('bias', [C, S * 2], f32, kind='ExternalInput').ap()
    lin_d = nc.dram_tensor('lin', [C, 2 * C], bf16, kind='ExternalInput').ap()
    # transposed output: [c, (tile, z, node)]; host un-permutes
    out_d = nc.dram_tensor('out', [C, TPC * 4 * TILE_N], f32, kind='ExternalOutput').ap()

    TD = D * TILE_N       # 1152: one tile's x block
    TV = NM55 * TILE_N    # 7040: one tile's V block

    with TileContext(nc) as tc:
        with (
            tc.tile_pool(name='const', bufs=1) as constp,
            tc.tile_pool(name='xt', bufs=1) as xtp,
            tc.tile_pool(name='vb', bufs=1) as vbp,
            tc.tile_pool(name='tg', bufs=1) as tgp,
            tc.tile_pool(name='ff', bufs=1) as ffp,
            tc.tile_pool(name='sb', bufs=1) as sbp,
            tc.tile_pool(name='facc', bufs=1) as faccp,
            tc.tile_pool(name='outt', bufs=1) as outp,
            tc.tile_pool(name='ps_misc', bufs=2, space='PSUM') as ps_m,
        ):
            wc_s = constp.tile([C, S * NM55 * 36], bf16)
            nc.sync.dma_start(out=wc_s[:], in_=wc_d[:])
            gk_s = constp.tile([C, S * 2 * C], bf16)
            nc.sync.dma_start(out=gk_s[:], in_=gk_d[:])
            bias_s = constp.tile([C, S * 2], f32)
            nc.sync.dma_start(out=bias_s[:], in_=bias_d[:])
            lin_s = constp.tile([C, 2 * C], bf16)
            nc.sync.dma_start(out=lin_s[:], in_=lin_d[:])

            for rep in range(repeat):
                xt_t = xtp.tile([C, TPC * TD], bf16)
                nc.sync.dma_start(out=xt_t[:], in_=xt_d[:])

                for h in range(TPC // 2):      # tile pairs (t = 2h, 2h+1)
                    # ---- monomials for both tiles: vb2[c, (t2, m, n)] ----
                    vb2 = vbp.tile([C, 2 * TV], bf16)
                    xb2 = (xt_t[:, 2 * h * TD:(2 * h + 2) * TD]
                           .rearrange('p (t inn) -> p t inn', t=2))
                    vb2v = vb2[:, :].rearrange('p (t mn) -> p t mn', t=2)
                    for o in range(D):
                        nj = D - o
                        nc.vector.tensor_mul(
                            vb2v[:, :, OSTART[o] * TILE_N:(OSTART[o] + nj) * TILE_N],
                            xb2[:, :, 0:nj * TILE_N],
                            xb2[:, :, o * TILE_N:(o + nj) * TILE_N])
                    nc.vector.tensor_copy(
                        vb2v[:, :, 45 * TILE_N:54 * TILE_N], xb2)
                    nc.vector.memset(
                        vb2v[:, :, 54 * TILE_N:55 * TILE_N], 1.0)

                    if stages < 2:
                        ot = outp.tile([C, 2 * 4 * TILE_N], f32, tag='ot')
                        nc.vector.tensor_copy(ot[:, 0:TILE_N], vb2[:, 0:TILE_N])
                        nc.vector.memset(ot[:, TILE_N:], 0.0)
                        nc.sync.dma_start(
                            out=out_d[:, 2 * h * 4 * TILE_N:(2 * h + 2) * 4 * TILE_N],
                            in_=ot[:])
                        continue

                    ot = outp.tile([C, 2 * 4 * TILE_N], f32, tag='ot')
                    for t2 in range(2):
                        s = 2 * h + t2   # species == tile index
                        # ---- F[c,(z,n,i)] = sum_m V[c,(m,n)] wc[c,(s,m,zi)] ----
                        ff = ffp.tile([C, 4 * TILE_N * D], f32)
                        v4 = (vb2[:, t2 * TV:(t2 + 1) * TV]
                              .rearrange('p (m n) -> p n m', m=NM55).unsqueeze(1))
                        wz = wc_s[:, s * NM55 * 36:(s + 1) * NM55 * 36].rearrange(
                            'p (m zi) -> p zi m', zi=36)
                        ff4 = ff[:, :].rearrange('p (z n i) -> p z i n', z=4, n=TILE_N)
                        for z in range(4):
                            for i0, ni in ((0, 5), (5, 4)):
                                tg = tgp.tile([C, 5 * TILE_N * NM55], bf16)
                                tg_v = tg[:, 0:ni * TILE_N * NM55].rearrange(
                                    'p (i n m) -> p i n m', i=ni, n=TILE_N)
                                nc.vector.tensor_mul(
                                    tg_v,
                                    v4.broadcast_to([C, ni, TILE_N, NM55]),
                                    wz[:, z * 9 + i0:z * 9 + i0 + ni, :]
                                    .unsqueeze(2).broadcast_to([C, ni, TILE_N, NM55]))
                                nc.vector.tensor_reduce(
                                    ff4[:, z, i0:i0 + ni, :], tg_v,
                                    axis=mybir.AxisListType.X, op=AL.add)

                        # ---- f[c,(z,n)] = sum_i F[c,(z,n,i)] * x[c,(i,n)] ----
                        gg = sbp.tile([C, 4 * TILE_N * D], bf16, tag='gg')
                        xv = (xt_t[:, s * TD:(s + 1) * TD]
                              .rearrange('p (i n) -> p n i', i=D)
                              .unsqueeze(1).broadcast_to([C, 4, TILE_N, D]))
                        nc.vector.tensor_mul(
                            gg[:, :].rearrange('p (z n i) -> p z n i', z=4, n=TILE_N),
                            ff[:, :].rearrange('p (z n i) -> p z n i', z=4, n=TILE_N),
                            xv)
                        facc = faccp.tile([C, 4 * TILE_N], f32)
                        nc.vector.tensor_reduce(
                            facc[:],
                            gg[:, :].rearrange('p (zn i) -> p zn i', i=D),
                            axis=mybir.AxisListType.X, op=AL.add)

                        if stages < 5:
                            nc.vector.tensor_copy(
                                ot[:, t2 * 4 * TILE_N:(t2 + 1) * 4 * TILE_N], facc[:])
                            continue

                        # ---- gate matmuls: gate_half^T = gk_half^T @ f0^T ----
                        fTb = sbp.tile([C, TILE_N], bf16, tag='fTb')
                        nc.vector.tensor_copy(fTb[:], facc[:, 0:TILE_N])
                        gps = ps_m.tile([C, 2 * TILE_N], f32, tag='misc')
                        nc.tensor.matmul(gps[:, 0:TILE_N],
                                         gk_s[:, s * 256:s * 256 + 128], fTb[:],
                                         start=True, stop=True)
                        nc.tensor.matmul(gps[:, TILE_N:2 * TILE_N],
                                         gk_s[:, s * 256 + 128:s * 256 + 256], fTb[:],
                                         start=True, stop=True)

                        # ---- fused bias + gating: fg = (gps + bias) * facc ----
                        fg = sbp.tile([C, 4 * TILE_N], bf16, tag='fg')
                        nc.vector.scalar_tensor_tensor(
                            out=fg[:, 0:TILE_N],
                            in0=gps[:, 0:TILE_N],
                            scalar=bias_s[:, 2 * s:2 * s + 1],
                            in1=facc[:, 0:TILE_N],
                            op0=AL.add, op1=AL.mult)
                        nc.vector.scalar_tensor_tensor(
                            out=fg[:, TILE_N:].rearrange('p (zz n) -> p zz n', zz=3),
                            in0=gps[:, TILE_N:2 * TILE_N].unsqueeze(1)
                            .broadcast_to([C, 3, TILE_N]),
                            scalar=bias_s[:, 2 * s + 1:2 * s + 2],
                            in1=facc[:, TILE_N:].rearrange('p (zz n) -> p zz n', zz=3),
                            op0=AL.add, op1=AL.mult)

                        # ---- linear (c-major): out^T [k, (z, n)] ----
                        ops_ = ps_m.tile([C, 4 * TILE_N], f32, tag='misc')
                        nc.tensor.matmul(ops_[:, 0:TILE_N], lin_s[:, 0:C],
                                         fg[:, 0:TILE_N], start=True, stop=True)
                        nc.tensor.matmul(ops_[:, TILE_N:4 * TILE_N], lin_s[:, C:2 * C],
                                         fg[:, TILE_N:4 * TILE_N], start=True, stop=True)
                        nc.vector.tensor_copy(
                            ot[:, t2 * 4 * TILE_N:(t2 + 1) * 4 * TILE_N], ops_[:])

                    nc.sync.dma_start(
                        out=out_d[:, 2 * h * 4 * TILE_N:(2 * h + 2) * 4 * TILE_N],
                        in_=ot[:])

    nc.compile()
    return nc


_NC_CACHE = {}


def _get_device(repeat=1, stages=5):
    key = (repeat, stages)
    if key not in _NC_CACHE:
        _NC_CACHE[key] = build_device(repeat, stages)
    return _NC_CACHE[key]


def kernel(**inputs):
    from concourse.bass_utils import run_bass_kernel_spmd

    in_maps, device_rows, overflow_idx = host_prepare(inputs)
    nc = _get_device(1)
    res = run_bass_kernel_spmd(nc, in_maps, list(range(N_CORES)))

    ntot = np.asarray(inputs['node_species']).shape[0]
    out = np.zeros((ntot, 4 * C), np.float32)
    for k in range(N_CORES):
        rows = device_rows[k]
        valid = rows >= 0
        # device output is [c, (tile, z, node)]; un-permute to [node, 512]
        a = res.results[k]['out'].reshape(C, TPC, 4, TILE_N)
        o = np.empty((NODES_PER_CORE, 4 * C), np.float32)
        o[:, :C] = a[:, :, 0, :].transpose(1, 2, 0).reshape(NODES_PER_CORE, C)
        o[:, C:] = a[:, :, 1:4, :].transpose(1, 3, 0, 2).reshape(NODES_PER_CORE, 3 * C)
        out[rows[valid]] = o[valid]
    if len(overflow_idx):
        out[overflow_idx] = _numpy_forward(inputs, overflow_idx)
    return out


# revision 7
# speedup vs baseline: 12.0763x; 1.4608x over previous
"""Trainium2 Bass kernel for nn_EquivariantProductBasisBlock (MACE product-basis block).

Self-contained: host-side sharding/preprocessing + Bass/Tile device kernel on 8 cores.

Math (validated vs reference): per node n, channel c, species s, x = x[n,c,:] in R^9:
    out[z] = sum_i x_i * F[z,i],   F = C1 + C2 @ x + C3h @ y,   y = {x_j x_k}_{j<=k}
with C* the species/channel coefficient tables folded from (u*, w*) on the host.
Then gate = f0 @ gate_kernel[s] + gate_bias[s]; f0*=gate[:C]; f1*=gate[C:];
out = [f0 @ lin0, f1 @ lin1] / sqrt(C).

Device layout: channels on SBUF partitions, nodes species-sorted so every tile is
species-pure (tile t = species t, 10 tiles/core). The runtime bills ~40-50us per
instruction regardless of engine or size, so the program minimizes instruction
count: one bulk input DMA, monomial build batched over tile pairs (3D APs),
the per-tile V@W contraction in 8 broadcast-mul + 8 reduce ops (SBUF-bound
granularity), fused bias+gating via scalar_tensor_tensor, and per-pair output DMA.

Species overflow beyond 1024 nodes/species is computed on the host in numpy.
"""

import numpy as np

N_CORES = 8
C, D, S = 128, 9, 10
NM = 45           # deg-2 monomials
NROW = 56         # host slots: 45 y | 9 x | 1 one | 1 pad (overflow path)
NM55 = 55         # device slots per channel: 45 y | 9 x | 1 one
TILE_N = 128
TPC = S           # tiles per core (one per species)
NODES_PER_CORE = TPC * TILE_N          # 1280
CAP_PER_SPECIES = N_CORES * TILE_N     # 1024 device-handled nodes per species

# monomials ordered by diagonal offset o=k-j then j: slot(o,j) = OSTART[o]+j.
# Each V-build op is then pure step-1 (out/in0/in1 all contiguous runs).
OSTART = [0] * D
for o in range(1, D):
    OSTART[o] = OSTART[o - 1] + (D - (o - 1))
MONO_JK = [(j, j + o) for o in range(D) for j in range(D - o)]


# ----------------------------------------------------------------------------
# host math
# ----------------------------------------------------------------------------

def _build_xr(node_feats):
    n = node_feats.shape[0]
    x = np.empty((n, C, D), np.float32)
    x[:, :, 0] = node_feats[:, :C]
    x[:, :, 1:4] = node_feats[:, C:4 * C].reshape(n, C, 3)
    x[:, :, 4:9] = node_feats[:, 4 * C:].reshape(n, C, 5)
    return x


def _build_coeff_tables(i):
    def c3h(u3, w3):
        c3 = np.einsum('zijkp,spc->sczijk', u3, w3[:, :, :], optimize=True)
        out = np.zeros(c3.shape[:4] + (NM,), np.float64)
        for m, (j, k) in enumerate(MONO_JK):
            out[..., m] = c3[..., j, k] if j == k else c3[..., j, k] + c3[..., k, j]
        return out

    def c2(u2, w2):
        return np.einsum('zijp,spc->sczij', u2, w2, optimize=True)

    def c1(u1, w1):
        return np.einsum('zip,spc->sczi', u1, w1, optimize=True)

    h0 = c3h(i['u3_0e'], i['w3_0e']); h1 = c3h(i['u3_1o'], i['w3_1o'])
    q0 = c2(i['u2_0e'], i['w2_0e']);  q1 = c2(i['u2_1o'], i['w2_1o'])
    l0 = c1(i['u1_0e'], i['w1_0e']);  l1 = c1(i['u1_1o'], i['w1_1o'])

    W = np.zeros((S, C, NROW, 36), np.float64)
    W[:, :, 0:45, 0:9] = np.moveaxis(h0[:, :, 0], -1, -2)
    W[:, :, 45:54, 0:9] = np.moveaxis(q0[:, :, 0], -1, -2)
    W[:, :, 54, 0:9] = l0[:, :, 0]
    for z in range(3):
        sl = slice(9 + z * 9, 18 + z * 9)
        W[:, :, 0:45, sl] = np.moveaxis(h1[:, :, z], -1, -2)
        W[:, :, 45:54, sl] = np.moveaxis(q1[:, :, z], -1, -2)
        W[:, :, 54, sl] = l1[:, :, z]
    return W.astype(np.float32)   # [S, C, 56, 36]


def _numpy_forward(inputs, idx):
    """Reference-equivalent host computation for node subset idx (overflow path)."""
    i = {k: np.asarray(v) for k, v in inputs.items()}
    nf = i['node_feats'][idx]; sp = i['node_species'][idx]
    xr = _build_xr(nf)
    W = _build_coeff_tables(i)
    n = nf.shape[0]
    V = np.empty((n, C, NROW), np.float32)
    for m, (j, k) in enumerate(MONO_JK):
        V[:, :, m] = xr[:, :, j] * xr[:, :, k]
    V[:, :, 45:54] = xr
    V[:, :, 54] = 1.0
    V[:, :, 55] = 0.0
    F = np.einsum('ncm,ncmz->ncz', V, W[sp], optimize=True)
    f = np.einsum('nczi,nci->ncz', F.reshape(n, C, 4, D), xr, optimize=True)
    f0, f1 = f[:, :, 0], f[:, :, 1:4]
    gate = np.einsum('nc,nck->nk', f0, i['gate_kernel'][sp], optimize=True) + i['gate_bias'][sp]
    f0g = f0 * gate[:, :C]
    f1g = f1 * gate[:, C:, None]
    inv = 1.0 / np.sqrt(np.float32(C))
    o0 = np.einsum('nc,ck->nk', f0g, i['lin_w_0e'], optimize=True) * inv
    o1 = np.einsum('ncd,ck->nkd', f1g, i['lin_w_1o'], optimize=True) * inv
    return np.concatenate([o0.reshape(n, C), o1.reshape(n, C * 3)], axis=1).astype(np.float32)


def _bf16(x):
    import ml_dtypes
    return np.asarray(x, np.float32).astype(ml_dtypes.bfloat16)


def host_prepare(inputs):
    """Returns (per_core_inmaps, device_rows [N_CORES,1280] global node ids (-1 pad),
    overflow_idx)."""
    i = {k: np.asarray(v) for k, v in inputs.items()}
    sp = i['node_species']

    order = np.argsort(sp, kind='stable')
    sorted_sp = sp[order]
    device_rows = np.full((N_CORES, NODES_PER_CORE), -1, np.int64)
    overflow = []
    for s in range(S):
        ids = order[sorted_sp == s]
        dev = ids[:CAP_PER_SPECIES]
        overflow.append(ids[CAP_PER_SPECIES:])
        for k in range(N_CORES):
            chunk = dev[k * TILE_N:(k + 1) * TILE_N]
            device_rows[k, s * TILE_N: s * TILE_N + len(chunk)] = chunk
    overflow_idx = np.concatenate(overflow) if overflow else np.zeros(0, np.int64)

    xr = _build_xr(i['node_feats'])                       # [N, C, 9]
    W = _build_coeff_tables(i)                            # [S, C, 56, 36]
    # channel-major coefficient table: wc[c, (s, m55, zi36)]
    wc_bf = _bf16(np.ascontiguousarray(
        W[:, :, :55, :].transpose(1, 0, 2, 3).reshape(C, S * 55 * 36)))

    gk = np.zeros((C, S * 2 * C), np.float32)             # rows c, col s*256 + j
    for s in range(S):
        gk[:, s * 256:(s + 1) * 256] = i['gate_kernel'][s]

    bias = np.zeros((C, S * 2), np.float32)               # rows k2%128, col s*2 + half
    for s in range(S):
        bias[:, 2 * s] = i['gate_bias'][s, :C]
        bias[:, 2 * s + 1] = i['gate_bias'][s, C:]

    inv = 1.0 / np.sqrt(np.float32(C))
    lin = np.concatenate([i['lin_w_0e'] * inv, i['lin_w_1o'] * inv], axis=1)  # [128, 256]

    gk_bf = _bf16(gk); lin_bf = _bf16(lin)

    in_maps = []
    for k in range(N_CORES):
        rows = device_rows[k]
        xr_core = np.zeros((NODES_PER_CORE, C * D), np.float32)
        valid = rows >= 0
        xr_core[valid] = xr[rows[valid]].reshape(-1, C * D)
        # channel-major x: xt[c, (tile, i, node)]
        xt = xr_core.reshape(TPC, TILE_N, C, D).transpose(2, 0, 3, 1)
        in_maps.append({
            'xt': _bf16(np.ascontiguousarray(xt.reshape(C, TPC * D * TILE_N))),
            'wc': wc_bf,
            'gk': gk_bf,
            'bias': bias,
            'lin': lin_bf,
        })
    return in_maps, device_rows, overflow_idx


# ----------------------------------------------------------------------------
# device kernel
# ----------------------------------------------------------------------------

def build_device(repeat=1, stages=5):
    import concourse.bacc as bacc
    import concourse.mybir as mybir
    from concourse.tile import TileContext

    f32, bf16 = mybir.dt.float32, mybir.dt.bfloat16
    AL = mybir.AluOpType

    nc = bacc.Bacc("TRN2", target_bir_lowering=False, debug=False,
                   num_devices=N_CORES)

    xt_d = nc.dram_tensor('xt', [C, TPC * D * TILE_N], bf16, kind='ExternalInput').ap()
    wc_d = nc.dram_tensor('wc', [C, S * NM55 * 36], bf16, kind='ExternalInput').ap()
    gk_d = nc.dram_tensor('gk', [C, S * 2 * C], bf16, kind='ExternalInput').ap()
    bias_d = nc.dram_tensor('bias', [C, S * 2], f32, kind='ExternalInput').ap()
    lin_d = nc.dram_tensor('lin', [C, 2 * C], bf16, kind='ExternalInput').ap()
    # transposed output: [c, (tile, z, node)]; host un-permutes
    out_d = nc.dram_tensor('out', [C, TPC * 4 * TILE_N], f32, kind='ExternalOutput').ap()

    TD = D * TILE_N       # 1152: one tile's x block
    TV = NM55 * TILE_N    # 7040: one tile's V block

    with TileContext(nc) as tc:
        with (
            tc.tile_pool(name='const', bufs=1) as constp,
            tc.tile_pool(name='xt', bufs=1) as xtp,
            tc.tile_pool(name='vb', bufs=1) as vbp,
            tc.tile_pool(name='tg', bufs=1) as tgp,
            tc.tile_pool(name='ff', bufs=1) as ffp,
            tc.tile_pool(name='sb', bufs=1) as sbp,
            tc.tile_pool(name='facc', bufs=1) as faccp,
            tc.tile_pool(name='outt', bufs=1) as outp,
            tc.tile_pool(name='ps_misc', bufs=2, space='PSUM') as ps_m,
        ):
            wc_s = constp.tile([C, S * NM55 * 36], bf16)
            nc.sync.dma_start(out=wc_s[:], in_=wc_d[:])
            gk_s = constp.tile([C, S * 2 * C], bf16)
            nc.sync.dma_start(out=gk_s[:], in_=gk_d[:])
            bias_s = constp.tile([C, S * 2], f32)
            nc.sync.dma_start(out=bias_s[:], in_=bias_d[:])
            lin_s = constp.tile([C, 2 * C], bf16)
            nc.sync.dma_start(out=lin_s[:], in_=lin_d[:])

            for rep in range(repeat):
                xt_t = xtp.tile([C, TPC * TD], bf16)
                nc.sync.dma_start(out=xt_t[:], in_=xt_d[:])

                for h in range(TPC // 2):      # tile pairs (t = 2h, 2h+1)
                    # ---- monomials for both tiles: vb2[c, (t2, m, n)] ----
                    vb2 = vbp.tile([C, 2 * TV], bf16)
                    xb2 = (xt_t[:, 2 * h * TD:(2 * h + 2) * TD]
                           .rearrange('p (t inn) -> p t inn', t=2))
                    vb2v = vb2[:, :].rearrange('p (t mn) -> p t mn', t=2)
                    for o in range(D):
                        nj = D - o
                        nc.vector.tensor_mul(
                            vb2v[:, :, OSTART[o] * TILE_N:(OSTART[o] + nj) * TILE_N],
                            xb2[:, :, 0:nj * TILE_N],
                            xb2[:, :, o * TILE_N:(o + nj) * TILE_N])
                    nc.vector.tensor_copy(
                        vb2v[:, :, 45 * TILE_N:54 * TILE_N], xb2)
                    nc.vector.memset(
                        vb2v[:, :, 54 * TILE_N:55 * TILE_N], 1.0)

                    if stages < 2:
                        ot = outp.tile([C, 2 * 4 * TILE_N], f32, tag='ot')
                        nc.vector.tensor_copy(ot[:, 0:TILE_N], vb2[:, 0:TILE_N])
                        nc.vector.memset(ot[:, TILE_N:], 0.0)
                        nc.sync.dma_start(
                            out=out_d[:, 2 * h * 4 * TILE_N:(2 * h + 2) * 4 * TILE_N],
                            in_=ot[:])
                        continue

                    ot = outp.tile([C, 2 * 4 * TILE_N], f32, tag='ot')
                    for t2 in range(2):
                        s = 2 * h + t2   # species == tile index
                        # ---- F[c,(z,n,i)] = sum_m V[c,(m,n)] wc[c,(s,m,zi)] ----
                        ff = ffp.tile([C, 4 * TILE_N * D], f32)
                        v4 = (vb2[:, t2 * TV:(t2 + 1) * TV]
                              .rearrange('p (m n) -> p n m', m=NM55).unsqueeze(1))
                        wz = wc_s[:, s * NM55 * 36:(s + 1) * NM55 * 36].rearrange(
                            'p (m zi) -> p zi m', zi=36)
                        ff4 = ff[:, :].rearrange('p (z n i) -> p z i n', z=4, n=TILE_N)
                        for z in range(4):
                            for i0, ni in ((0, 5), (5, 4)):
                                tg = tgp.tile([C, 5 * TILE_N * NM55], bf16)
                                tg_v = tg[:, 0:ni * TILE_N * NM55].rearrange(
                                    'p (i n m) -> p i n m', i=ni, n=TILE_N)
                                nc.vector.tensor_mul(
                                    tg_v,
                                    v4.broadcast_to([C, ni, TILE_N, NM55]),
                                    wz[:, z * 9 + i0:z * 9 + i0 + ni, :]
                                    .unsqueeze(2).broadcast_to([C, ni, TILE_N, NM55]))
                                nc.vector.tensor_reduce(
                                    ff4[:, z, i0:i0 + ni, :], tg_v,
                                    axis=mybir.AxisListType.X, op=AL.add)

                        # ---- f[c,(z,n)] = sum_i F[c,(z,n,i)] * x[c,(i,n)] ----
                        gg = sbp.tile([C, 4 * TILE_N * D], bf16, tag='gg')
                        xv = (xt_t[:, s * TD:(s + 1) * TD]
                              .rearrange('p (i n) -> p n i', i=D)
                              .unsqueeze(1).broadcast_to([C, 4, TILE_N, D]))
                        nc.vector.tensor_mul(
                            gg[:, :].rearrange('p (z n i) -> p z n i', z=4, n=TILE_N),
                            ff[:, :].rearrange('p (z n i) -> p z n i', z=4, n=TILE_N),
                            xv)
                        facc = faccp.tile([C, 4 * TILE_N], bf16)
                        with nc.allow_low_precision(reason='9-elem reduce'):
                            nc.vector.tensor_reduce(
                                facc[:],
                                gg[:, :].rearrange('p (zn i) -> p zn i', i=D),
                                axis=mybir.AxisListType.X, op=AL.add)

                        if stages < 5:
                            nc.vector.tensor_copy(
                                ot[:, t2 * 4 * TILE_N:(t2 + 1) * 4 * TILE_N], facc[:])
                            continue

                        # ---- gate matmuls: gate_half^T = gk_half^T @ f0^T ----
                        gps = ps_m.tile([C, 2 * TILE_N], f32, tag='misc')
                        nc.tensor.matmul(gps[:, 0:TILE_N],
                                         gk_s[:, s * 256:s * 256 + 128],
                                         facc[:, 0:TILE_N],
                                         start=True, stop=True)
                        nc.tensor.matmul(gps[:, TILE_N:2 * TILE_N],
                                         gk_s[:, s * 256 + 128:s * 256 + 256],
                                         facc[:, 0:TILE_N],
                                         start=True, stop=True)

                        # ---- fused bias + gating: fg = (gps + bias) * facc ----
                        fg = sbp.tile([C, 4 * TILE_N], bf16, tag='fg')
                        nc.vector.scalar_tensor_tensor(
                            out=fg[:, 0:TILE_N],
                            in0=gps[:, 0:TILE_N],
                            scalar=bias_s[:, 2 * s:2 * s + 1],
                            in1=facc[:, 0:TILE_N],
                            op0=AL.add, op1=AL.mult)
                        nc.vector.scalar_tensor_tensor(
                            out=fg[:, TILE_N:].rearrange('p (zz n) -> p zz n', zz=3),
                            in0=gps[:, TILE_N:2 * TILE_N].unsqueeze(1)
                            .broadcast_to([C, 3, TILE_N]),
                            scalar=bias_s[:, 2 * s + 1:2 * s + 2],
                            in1=facc[:, TILE_N:].rearrange('p (zz n) -> p zz n', zz=3),
                            op0=AL.add, op1=AL.mult)

                        # ---- linear (c-major): out^T [k, (z, n)] ----
                        ops_ = ps_m.tile([C, 4 * TILE_N], f32, tag='misc')
                        nc.tensor.matmul(ops_[:, 0:TILE_N], lin_s[:, 0:C],
                                         fg[:, 0:TILE_N], start=True, stop=True)
                        nc.tensor.matmul(ops_[:, TILE_N:4 * TILE_N], lin_s[:, C:2 * C],
                                         fg[:, TILE_N:4 * TILE_N], start=True, stop=True)
                        nc.vector.tensor_copy(
                            ot[:, t2 * 4 * TILE_N:(t2 + 1) * 4 * TILE_N], ops_[:])
                    nc.sync.dma_start(
                        out=out_d[:, 2 * h * 4 * TILE_N:(2 * h + 2) * 4 * TILE_N],
                        in_=ot[:])

    nc.compile()
    return nc


_NC_CACHE = {}


def _get_device(repeat=1, stages=5):
    key = (repeat, stages)
    if key not in _NC_CACHE:
        _NC_CACHE[key] = build_device(repeat, stages)
    return _NC_CACHE[key]


def kernel(**inputs):
    from concourse.bass_utils import run_bass_kernel_spmd

    in_maps, device_rows, overflow_idx = host_prepare(inputs)
    nc = _get_device(1)
    res = run_bass_kernel_spmd(nc, in_maps, list(range(N_CORES)))

    ntot = np.asarray(inputs['node_species']).shape[0]
    out = np.zeros((ntot, 4 * C), np.float32)
    for k in range(N_CORES):
        rows = device_rows[k]
        valid = rows >= 0
        # device output is [c, (tile, z, node)]; un-permute to [node, 512]
        a = res.results[k]['out'].reshape(C, TPC, 4, TILE_N)
        o = np.empty((NODES_PER_CORE, 4 * C), np.float32)
        o[:, :C] = a[:, :, 0, :].transpose(1, 2, 0).reshape(NODES_PER_CORE, C)
        o[:, C:] = a[:, :, 1:4, :].transpose(1, 3, 0, 2).reshape(NODES_PER_CORE, 3 * C)
        out[rows[valid]] = o[valid]
    if len(overflow_idx):
        out[overflow_idx] = _numpy_forward(inputs, overflow_idx)
    return out


# revision 8
# speedup vs baseline: 12.2126x; 1.0113x over previous
"""Trainium2 Bass kernel for nn_EquivariantProductBasisBlock (MACE product-basis block).

Self-contained: host-side sharding/preprocessing + Bass/Tile device kernel on 8 cores.

Math (validated vs reference): per node n, channel c, species s, x = x[n,c,:] in R^9:
    out[z] = sum_i x_i * F[z,i],   F = C1 + C2 @ x + C3h @ y,   y = {x_j x_k}_{j<=k}
with C* the species/channel coefficient tables folded from (u*, w*) on the host.
Then gate = f0 @ gate_kernel[s] + gate_bias[s]; f0*=gate[:C]; f1*=gate[C:];
out = [f0 @ lin0, f1 @ lin1] / sqrt(C).

Device layout: channels on SBUF partitions, nodes species-sorted so every tile is
species-pure (tile t = species t, 10 tiles/core). The runtime bills ~40-50us per
instruction regardless of engine or size, so the program minimizes instruction
count: one bulk input DMA, monomial build batched over tile pairs (3D APs),
the per-tile V@W contraction in 8 broadcast-mul + 8 reduce ops (SBUF-bound
granularity), fused bias+gating via scalar_tensor_tensor, and per-pair output DMA.

Species overflow beyond 1024 nodes/species is computed on the host in numpy.
"""

import numpy as np

N_CORES = 8
C, D, S = 128, 9, 10
NM = 45           # deg-2 monomials
NROW = 56         # host slots: 45 y | 9 x | 1 one | 1 pad (overflow path)
NM55 = 55         # device slots per channel: 45 y | 9 x | 1 one
TILE_N = 128
TPC = S           # tiles per core (one per species)
NODES_PER_CORE = TPC * TILE_N          # 1280
CAP_PER_SPECIES = N_CORES * TILE_N     # 1024 device-handled nodes per species

# monomials ordered by diagonal offset o=k-j then j: slot(o,j) = OSTART[o]+j.
# Each V-build op is then pure step-1 (out/in0/in1 all contiguous runs).
OSTART = [0] * D
for o in range(1, D):
    OSTART[o] = OSTART[o - 1] + (D - (o - 1))
MONO_JK = [(j, j + o) for o in range(D) for j in range(D - o)]


# ----------------------------------------------------------------------------
# host math
# ----------------------------------------------------------------------------

def _build_xr(node_feats):
    n = node_feats.shape[0]
    x = np.empty((n, C, D), np.float32)
    x[:, :, 0] = node_feats[:, :C]
    x[:, :, 1:4] = node_feats[:, C:4 * C].reshape(n, C, 3)
    x[:, :, 4:9] = node_feats[:, 4 * C:].reshape(n, C, 5)
    return x


def _build_coeff_tables(i):
    def c3h(u3, w3):
        c3 = np.einsum('zijkp,spc->sczijk', u3, w3[:, :, :], optimize=True)
        out = np.zeros(c3.shape[:4] + (NM,), np.float64)
        for m, (j, k) in enumerate(MONO_JK):
            out[..., m] = c3[..., j, k] if j == k else c3[..., j, k] + c3[..., k, j]
        return out

    def c2(u2, w2):
        return np.einsum('zijp,spc->sczij', u2, w2, optimize=True)

    def c1(u1, w1):
        return np.einsum('zip,spc->sczi', u1, w1, optimize=True)

    h0 = c3h(i['u3_0e'], i['w3_0e']); h1 = c3h(i['u3_1o'], i['w3_1o'])
    q0 = c2(i['u2_0e'], i['w2_0e']);  q1 = c2(i['u2_1o'], i['w2_1o'])
    l0 = c1(i['u1_0e'], i['w1_0e']);  l1 = c1(i['u1_1o'], i['w1_1o'])

    W = np.zeros((S, C, NROW, 36), np.float64)
    W[:, :, 0:45, 0:9] = np.moveaxis(h0[:, :, 0], -1, -2)
    W[:, :, 45:54, 0:9] = np.moveaxis(q0[:, :, 0], -1, -2)
    W[:, :, 54, 0:9] = l0[:, :, 0]
    for z in range(3):
        sl = slice(9 + z * 9, 18 + z * 9)
        W[:, :, 0:45, sl] = np.moveaxis(h1[:, :, z], -1, -2)
        W[:, :, 45:54, sl] = np.moveaxis(q1[:, :, z], -1, -2)
        W[:, :, 54, sl] = l1[:, :, z]
    return W.astype(np.float32)   # [S, C, 56, 36]


def _numpy_forward(inputs, idx):
    """Reference-equivalent host computation for node subset idx (overflow path)."""
    i = {k: np.asarray(v) for k, v in inputs.items()}
    nf = i['node_feats'][idx]; sp = i['node_species'][idx]
    xr = _build_xr(nf)
    W = _build_coeff_tables(i)
    n = nf.shape[0]
    V = np.empty((n, C, NROW), np.float32)
    for m, (j, k) in enumerate(MONO_JK):
        V[:, :, m] = xr[:, :, j] * xr[:, :, k]
    V[:, :, 45:54] = xr
    V[:, :, 54] = 1.0
    V[:, :, 55] = 0.0
    F = np.einsum('ncm,ncmz->ncz', V, W[sp], optimize=True)
    f = np.einsum('nczi,nci->ncz', F.reshape(n, C, 4, D), xr, optimize=True)
    f0, f1 = f[:, :, 0], f[:, :, 1:4]
    gate = np.einsum('nc,nck->nk', f0, i['gate_kernel'][sp], optimize=True) + i['gate_bias'][sp]
    f0g = f0 * gate[:, :C]
    f1g = f1 * gate[:, C:, None]
    inv = 1.0 / np.sqrt(np.float32(C))
    o0 = np.einsum('nc,ck->nk', f0g, i['lin_w_0e'], optimize=True) * inv
    o1 = np.einsum('ncd,ck->nkd', f1g, i['lin_w_1o'], optimize=True) * inv
    return np.concatenate([o0.reshape(n, C), o1.reshape(n, C * 3)], axis=1).astype(np.float32)


def _bf16(x):
    import ml_dtypes
    return np.asarray(x, np.float32).astype(ml_dtypes.bfloat16)


def host_prepare(inputs):
    """Returns (per_core_inmaps, device_rows [N_CORES,1280] global node ids (-1 pad),
    overflow_idx)."""
    i = {k: np.asarray(v) for k, v in inputs.items()}
    sp = i['node_species']

    order = np.argsort(sp, kind='stable')
    sorted_sp = sp[order]
    device_rows = np.full((N_CORES, NODES_PER_CORE), -1, np.int64)
    overflow = []
    for s in range(S):
        ids = order[sorted_sp == s]
        dev = ids[:CAP_PER_SPECIES]
        overflow.append(ids[CAP_PER_SPECIES:])
        for k in range(N_CORES):
            chunk = dev[k * TILE_N:(k + 1) * TILE_N]
            device_rows[k, s * TILE_N: s * TILE_N + len(chunk)] = chunk
    overflow_idx = np.concatenate(overflow) if overflow else np.zeros(0, np.int64)

    xr = _build_xr(i['node_feats'])                       # [N, C, 9]
    W = _build_coeff_tables(i)                            # [S, C, 56, 36]
    # channel-major coefficient table: wc[c, (s, m55, zi36)]
    wc_bf = _bf16(np.ascontiguousarray(
        W[:, :, :55, :].transpose(1, 0, 2, 3).reshape(C, S * 55 * 36)))

    gk = np.zeros((C, S * 2 * C), np.float32)             # rows c, col s*256 + j
    for s in range(S):
        gk[:, s * 256:(s + 1) * 256] = i['gate_kernel'][s]

    bias = np.zeros((C, S * 2), np.float32)               # rows k2%128, col s*2 + half
    for s in range(S):
        bias[:, 2 * s] = i['gate_bias'][s, :C]
        bias[:, 2 * s + 1] = i['gate_bias'][s, C:]

    inv = 1.0 / np.sqrt(np.float32(C))
    lin = np.concatenate([i['lin_w_0e'] * inv, i['lin_w_1o'] * inv], axis=1)  # [128, 256]

    gk_bf = _bf16(gk); lin_bf = _bf16(lin)

    in_maps = []
    for k in range(N_CORES):
        rows = device_rows[k]
        xr_core = np.zeros((NODES_PER_CORE, C * D), np.float32)
        valid = rows >= 0
        xr_core[valid] = xr[rows[valid]].reshape(-1, C * D)
        # channel-major x: xt[c, (tile, i, node)]
        xt = xr_core.reshape(TPC, TILE_N, C, D).transpose(2, 0, 3, 1)
        in_maps.append({
            'xt': _bf16(np.ascontiguousarray(xt.reshape(C, TPC * D * TILE_N))),
            'wc': wc_bf,
            'gk': gk_bf,
            'bias': bias,
            'lin': lin_bf,
        })
    return in_maps, device_rows, overflow_idx


# ----------------------------------------------------------------------------
# device kernel
# ----------------------------------------------------------------------------

def build_device(repeat=1, stages=5):
    import concourse.bacc as bacc
    import concourse.mybir as mybir
    from concourse.tile import TileContext

    f32, bf16 = mybir.dt.float32, mybir.dt.bfloat16
    AL = mybir.AluOpType

    nc = bacc.Bacc("TRN2", target_bir_lowering=False, debug=False,
                   num_devices=N_CORES)

    xt_d = nc.dram_tensor('xt', [C, TPC * D * TILE_N], bf16, kind='ExternalInput').ap()
    wc_d = nc.dram_tensor('wc', [C, S * NM55 * 36], bf16, kind='ExternalInput').ap()
    gk_d = nc.dram_tensor('gk', [C, S * 2 * C], bf16, kind='ExternalInput').ap()
    bias_d = nc.dram_tensor('bias', [C, S * 2], f32, kind='ExternalInput').ap()
    lin_d = nc.dram_tensor('lin', [C, 2 * C], bf16, kind='ExternalInput').ap()
    # transposed output: [c, (tile, z, node)]; host un-permutes
    out_d = nc.dram_tensor('out', [C, TPC * 4 * TILE_N], f32, kind='ExternalOutput').ap()

    TD = D * TILE_N       # 1152: one tile's x block
    TV = NM55 * TILE_N    # 7040: one tile's V block

    with TileContext(nc) as tc:
        with (
            tc.tile_pool(name='const', bufs=1) as constp,
            tc.tile_pool(name='xt', bufs=1) as xtp,
            tc.tile_pool(name='vb', bufs=1) as vbp,
            tc.tile_pool(name='tg', bufs=1) as tgp,
            tc.tile_pool(name='ff', bufs=1) as ffp,
            tc.tile_pool(name='sb', bufs=1) as sbp,
            tc.tile_pool(name='facc', bufs=1) as faccp,
            tc.tile_pool(name='outt', bufs=1) as outp,
            tc.tile_pool(name='ps_misc', bufs=2, space='PSUM') as ps_m,
        ):
            wc_s = constp.tile([C, S * NM55 * 36], bf16)
            nc.sync.dma_start(out=wc_s[:], in_=wc_d[:])
            gk_s = constp.tile([C, S * 2 * C], bf16)
            nc.sync.dma_start(out=gk_s[:], in_=gk_d[:])
            bias_s = constp.tile([C, S * 2], f32)
            nc.sync.dma_start(out=bias_s[:], in_=bias_d[:])
            lin_s = constp.tile([C, 2 * C], bf16)
            nc.sync.dma_start(out=lin_s[:], in_=lin_d[:])

            for rep in range(repeat):
              for (t0, ng) in ((0, 6), (6, 4)):   # tile groups
                xt_t = xtp.tile([C, 6 * TD], bf16)
                nc.sync.dma_start(out=xt_t[:, 0:ng * TD],
                                  in_=xt_d[:, t0 * TD:(t0 + ng) * TD])

                for h in range(t0 // 2, (t0 + ng) // 2):  # pairs (t = 2h, 2h+1)
                    # ---- monomials for both tiles: vb2[c, (t2, m, n)] ----
                    vb2 = vbp.tile([C, 2 * TV], bf16)
                    xb2 = (xt_t[:, (2 * h - t0) * TD:(2 * h - t0 + 2) * TD]
                           .rearrange('p (t inn) -> p t inn', t=2))
                    vb2v = vb2[:, :].rearrange('p (t mn) -> p t mn', t=2)
                    for o in range(D):
                        nj = D - o
                        nc.vector.tensor_mul(
                            vb2v[:, :, OSTART[o] * TILE_N:(OSTART[o] + nj) * TILE_N],
                            xb2[:, :, 0:nj * TILE_N],
                            xb2[:, :, o * TILE_N:(o + nj) * TILE_N])
                    nc.vector.tensor_copy(
                        vb2v[:, :, 45 * TILE_N:54 * TILE_N], xb2)
                    nc.vector.memset(
                        vb2v[:, :, 54 * TILE_N:55 * TILE_N], 1.0)

                    if stages < 2:
                        ot = outp.tile([C, 2 * 4 * TILE_N], f32, tag='ot')
                        nc.vector.tensor_copy(ot[:, 0:TILE_N], vb2[:, 0:TILE_N])
                        nc.vector.memset(ot[:, TILE_N:], 0.0)
                        nc.sync.dma_start(
                            out=out_d[:, 2 * h * 4 * TILE_N:(2 * h + 2) * 4 * TILE_N],
                            in_=ot[:])
                        continue

                    ot = outp.tile([C, 2 * 4 * TILE_N], f32, tag='ot')
                    for t2 in range(2):
                        s = 2 * h + t2   # species == tile index
                        # ---- F[c,(z,n,i)] = sum_m V[c,(m,n)] wc[c,(s,m,zi)] ----
                        ff = ffp.tile([C, 4 * TILE_N * D], f32)
                        v4 = (vb2[:, t2 * TV:(t2 + 1) * TV]
                              .rearrange('p (m n) -> p n m', m=NM55).unsqueeze(1))
                        wz = wc_s[:, s * NM55 * 36:(s + 1) * NM55 * 36].rearrange(
                            'p (m zi) -> p zi m', zi=36)
                        ff36 = ff[:, :].rearrange('p (zi n) -> p zi n', zi=36)
                        for zc in range(6):
                            zi0 = zc * 6
                            tg = tgp.tile([C, 6 * TILE_N * NM55], bf16)
                            tg_v = tg[:, :].rearrange(
                                'p (zi n m) -> p zi n m', zi=6, n=TILE_N)
                            nc.vector.tensor_mul(
                                tg_v,
                                v4.broadcast_to([C, 6, TILE_N, NM55]),
                                wz[:, zi0:zi0 + 6, :]
                                .unsqueeze(2).broadcast_to([C, 6, TILE_N, NM55]))
                            nc.vector.tensor_reduce(
                                ff36[:, zi0:zi0 + 6, :], tg_v,
                                axis=mybir.AxisListType.X, op=AL.add)

                        # ---- f[c,(z,n)] = sum_i F[c,(z,n,i)] * x[c,(i,n)] ----
                        gg = sbp.tile([C, 4 * TILE_N * D], bf16, tag='gg')
                        xv = (xt_t[:, (s - t0) * TD:(s - t0 + 1) * TD]
                              .rearrange('p (i n) -> p i n', i=D)
                              .unsqueeze(1).broadcast_to([C, 4, D, TILE_N]))
                        nc.vector.tensor_mul(
                            gg[:, :].rearrange('p (z i n) -> p z i n', z=4, i=D),
                            ff[:, :].rearrange('p (z i n) -> p z i n', z=4, i=D),
                            xv)
                        facc = faccp.tile([C, 4 * TILE_N], bf16)
                        with nc.allow_low_precision(reason='9-elem reduce'):
                            nc.vector.tensor_reduce(
                                facc[:, :].rearrange('p (z n) -> p z n', z=4),
                                gg[:, :].rearrange('p (z i n) -> p z n i', z=4, i=D),
                                axis=mybir.AxisListType.X, op=AL.add)

                        if stages < 5:
                            nc.vector.tensor_copy(
                                ot[:, t2 * 4 * TILE_N:(t2 + 1) * 4 * TILE_N], facc[:])
                            continue

                        # ---- gate matmuls: gate_half^T = gk_half^T @ f0^T ----
                        gps = ps_m.tile([C, 2 * TILE_N], f32, tag='misc')
                        nc.tensor.matmul(gps[:, 0:TILE_N],
                                         gk_s[:, s * 256:s * 256 + 128],
                                         facc[:, 0:TILE_N],
                                         start=True, stop=True)
                        nc.tensor.matmul(gps[:, TILE_N:2 * TILE_N],
                                         gk_s[:, s * 256 + 128:s * 256 + 256],
                                         facc[:, 0:TILE_N],
                                         start=True, stop=True)

                        # ---- fused bias + gating: fg = (gps + bias) * facc ----
                        fg = sbp.tile([C, 4 * TILE_N], bf16, tag='fg')
                        nc.vector.scalar_tensor_tensor(
                            out=fg[:, 0:TILE_N],
                            in0=gps[:, 0:TILE_N],
                            scalar=bias_s[:, 2 * s:2 * s + 1],
                            in1=facc[:, 0:TILE_N],
                            op0=AL.add, op1=AL.mult)
                        nc.vector.scalar_tensor_tensor(
                            out=fg[:, TILE_N:].rearrange('p (zz n) -> p zz n', zz=3),
                            in0=gps[:, TILE_N:2 * TILE_N].unsqueeze(1)
                            .broadcast_to([C, 3, TILE_N]),
                            scalar=bias_s[:, 2 * s + 1:2 * s + 2],
                            in1=facc[:, TILE_N:].rearrange('p (zz n) -> p zz n', zz=3),
                            op0=AL.add, op1=AL.mult)

                        # ---- linear (c-major): out^T [k, (z, n)] ----
                        ops_ = ps_m.tile([C, 4 * TILE_N], f32, tag='misc')
                        nc.tensor.matmul(ops_[:, 0:TILE_N], lin_s[:, 0:C],
                                         fg[:, 0:TILE_N], start=True, stop=True)
                        nc.tensor.matmul(ops_[:, TILE_N:4 * TILE_N], lin_s[:, C:2 * C],
                                         fg[:, TILE_N:4 * TILE_N], start=True, stop=True)
                        nc.vector.tensor_copy(
                            ot[:, t2 * 4 * TILE_N:(t2 + 1) * 4 * TILE_N], ops_[:])
                    nc.sync.dma_start(
                        out=out_d[:, 2 * h * 4 * TILE_N:(2 * h + 2) * 4 * TILE_N],
                        in_=ot[:])

    nc.compile()
    return nc


_NC_CACHE = {}


def _get_device(repeat=1, stages=5):
    key = (repeat, stages)
    if key not in _NC_CACHE:
        _NC_CACHE[key] = build_device(repeat, stages)
    return _NC_CACHE[key]


def kernel(**inputs):
    from concourse.bass_utils import run_bass_kernel_spmd

    in_maps, device_rows, overflow_idx = host_prepare(inputs)
    nc = _get_device(1)
    res = run_bass_kernel_spmd(nc, in_maps, list(range(N_CORES)))

    ntot = np.asarray(inputs['node_species']).shape[0]
    out = np.zeros((ntot, 4 * C), np.float32)
    for k in range(N_CORES):
        rows = device_rows[k]
        valid = rows >= 0
        # device output is [c, (tile, z, node)]; un-permute to [node, 512]
        a = res.results[k]['out'].reshape(C, TPC, 4, TILE_N)
        o = np.empty((NODES_PER_CORE, 4 * C), np.float32)
        o[:, :C] = a[:, :, 0, :].transpose(1, 2, 0).reshape(NODES_PER_CORE, C)
        o[:, C:] = a[:, :, 1:4, :].transpose(1, 3, 0, 2).reshape(NODES_PER_CORE, 3 * C)
        out[rows[valid]] = o[valid]
    if len(overflow_idx):
        out[overflow_idx] = _numpy_forward(inputs, overflow_idx)
    return out


# revision 9
# speedup vs baseline: 12.2169x; 1.0004x over previous
"""Trainium2 Bass kernel for nn_EquivariantProductBasisBlock (MACE product-basis block).

Self-contained: host-side sharding/preprocessing + Bass/Tile device kernel on 8 cores.

Math (validated vs reference): per node n, channel c, species s, x = x[n,c,:] in R^9:
    out[z] = sum_i x_i * F[z,i],   F = C1 + C2 @ x + C3h @ y,   y = {x_j x_k}_{j<=k}
with C* the species/channel coefficient tables folded from (u*, w*) on the host.
Then gate = f0 @ gate_kernel[s] + gate_bias[s]; f0*=gate[:C]; f1*=gate[C:];
out = [f0 @ lin0, f1 @ lin1] / sqrt(C).

Device layout: channels on SBUF partitions, nodes species-sorted so every tile is
species-pure (tile t = species t, 10 tiles/core). The runtime bills ~40-50us per
instruction regardless of engine or size, so the program minimizes instruction
count: one bulk input DMA, monomial build batched over tile pairs (3D APs),
the per-tile V@W contraction in 8 broadcast-mul + 8 reduce ops (SBUF-bound
granularity), fused bias+gating via scalar_tensor_tensor, and per-pair output DMA.

Species overflow beyond 1024 nodes/species is computed on the host in numpy.
"""

import numpy as np

N_CORES = 8
C, D, S = 128, 9, 10
NM = 45           # deg-2 monomials
NROW = 56         # host slots: 45 y | 9 x | 1 one | 1 pad (overflow path)
NM55 = 55         # device slots per channel: 45 y | 9 x | 1 one
TILE_N = 128
TPC = S           # tiles per core (one per species)
NODES_PER_CORE = TPC * TILE_N          # 1280
CAP_PER_SPECIES = N_CORES * TILE_N     # 1024 device-handled nodes per species

# monomials ordered by diagonal offset o=k-j then j: slot(o,j) = OSTART[o]+j.
# Each V-build op is then pure step-1 (out/in0/in1 all contiguous runs).
OSTART = [0] * D
for o in range(1, D):
    OSTART[o] = OSTART[o - 1] + (D - (o - 1))
MONO_JK = [(j, j + o) for o in range(D) for j in range(D - o)]


# ----------------------------------------------------------------------------
# host math
# ----------------------------------------------------------------------------

def _build_xr(node_feats):
    n = node_feats.shape[0]
    x = np.empty((n, C, D), np.float32)
    x[:, :, 0] = node_feats[:, :C]
    x[:, :, 1:4] = node_feats[:, C:4 * C].reshape(n, C, 3)
    x[:, :, 4:9] = node_feats[:, 4 * C:].reshape(n, C, 5)
    return x


def _build_coeff_tables(i):
    def c3h(u3, w3):
        c3 = np.einsum('zijkp,spc->sczijk', u3, w3[:, :, :], optimize=True)
        out = np.zeros(c3.shape[:4] + (NM,), np.float64)
        for m, (j, k) in enumerate(MONO_JK):
            out[..., m] = c3[..., j, k] if j == k else c3[..., j, k] + c3[..., k, j]
        return out

    def c2(u2, w2):
        return np.einsum('zijp,spc->sczij', u2, w2, optimize=True)

    def c1(u1, w1):
        return np.einsum('zip,spc->sczi', u1, w1, optimize=True)

    h0 = c3h(i['u3_0e'], i['w3_0e']); h1 = c3h(i['u3_1o'], i['w3_1o'])
    q0 = c2(i['u2_0e'], i['w2_0e']);  q1 = c2(i['u2_1o'], i['w2_1o'])
    l0 = c1(i['u1_0e'], i['w1_0e']);  l1 = c1(i['u1_1o'], i['w1_1o'])

    W = np.zeros((S, C, NROW, 36), np.float64)
    W[:, :, 0:45, 0:9] = np.moveaxis(h0[:, :, 0], -1, -2)
    W[:, :, 45:54, 0:9] = np.moveaxis(q0[:, :, 0], -1, -2)
    W[:, :, 54, 0:9] = l0[:, :, 0]
    for z in range(3):
        sl = slice(9 + z * 9, 18 + z * 9)
        W[:, :, 0:45, sl] = np.moveaxis(h1[:, :, z], -1, -2)
        W[:, :, 45:54, sl] = np.moveaxis(q1[:, :, z], -1, -2)
        W[:, :, 54, sl] = l1[:, :, z]
    return W.astype(np.float32)   # [S, C, 56, 36]


def _numpy_forward(inputs, idx):
    """Reference-equivalent host computation for node subset idx (overflow path)."""
    i = {k: np.asarray(v) for k, v in inputs.items()}
    nf = i['node_feats'][idx]; sp = i['node_species'][idx]
    xr = _build_xr(nf)
    W = _build_coeff_tables(i)
    n = nf.shape[0]
    V = np.empty((n, C, NROW), np.float32)
    for m, (j, k) in enumerate(MONO_JK):
        V[:, :, m] = xr[:, :, j] * xr[:, :, k]
    V[:, :, 45:54] = xr
    V[:, :, 54] = 1.0
    V[:, :, 55] = 0.0
    F = np.einsum('ncm,ncmz->ncz', V, W[sp], optimize=True)
    f = np.einsum('nczi,nci->ncz', F.reshape(n, C, 4, D), xr, optimize=True)
    f0, f1 = f[:, :, 0], f[:, :, 1:4]
    gate = np.einsum('nc,nck->nk', f0, i['gate_kernel'][sp], optimize=True) + i['gate_bias'][sp]
    f0g = f0 * gate[:, :C]
    f1g = f1 * gate[:, C:, None]
    inv = 1.0 / np.sqrt(np.float32(C))
    o0 = np.einsum('nc,ck->nk', f0g, i['lin_w_0e'], optimize=True) * inv
    o1 = np.einsum('ncd,ck->nkd', f1g, i['lin_w_1o'], optimize=True) * inv
    return np.concatenate([o0.reshape(n, C), o1.reshape(n, C * 3)], axis=1).astype(np.float32)


def _bf16(x):
    import ml_dtypes
    return np.asarray(x, np.float32).astype(ml_dtypes.bfloat16)


def host_prepare(inputs):
    """Returns (per_core_inmaps, device_rows [N_CORES,1280] global node ids (-1 pad),
    overflow_idx)."""
    i = {k: np.asarray(v) for k, v in inputs.items()}
    sp = i['node_species']

    order = np.argsort(sp, kind='stable')
    sorted_sp = sp[order]
    device_rows = np.full((N_CORES, NODES_PER_CORE), -1, np.int64)
    overflow = []
    for s in range(S):
        ids = order[sorted_sp == s]
        dev = ids[:CAP_PER_SPECIES]
        overflow.append(ids[CAP_PER_SPECIES:])
        for k in range(N_CORES):
            chunk = dev[k * TILE_N:(k + 1) * TILE_N]
            device_rows[k, s * TILE_N: s * TILE_N + len(chunk)] = chunk
    overflow_idx = np.concatenate(overflow) if overflow else np.zeros(0, np.int64)

    xr = _build_xr(i['node_feats'])                       # [N, C, 9]
    W = _build_coeff_tables(i)                            # [S, C, 56, 36]
    # channel-major coefficient table: wc[c, (s, m55, zi36)]
    wc_bf = _bf16(np.ascontiguousarray(
        W[:, :, :55, :].transpose(1, 0, 2, 3).reshape(C, S * 55 * 36)))

    gk = np.zeros((C, S * 2 * C), np.float32)             # rows c, col s*256 + j
    for s in range(S):
        gk[:, s * 256:(s + 1) * 256] = i['gate_kernel'][s]

    bias = np.zeros((C, S * 2), np.float32)               # rows k2%128, col s*2 + half
    for s in range(S):
        bias[:, 2 * s] = i['gate_bias'][s, :C]
        bias[:, 2 * s + 1] = i['gate_bias'][s, C:]

    inv = 1.0 / np.sqrt(np.float32(C))
    lin = np.concatenate([i['lin_w_0e'] * inv, i['lin_w_1o'] * inv], axis=1)  # [128, 256]

    gk_bf = _bf16(gk); lin_bf = _bf16(lin)

    in_maps = []
    for k in range(N_CORES):
        rows = device_rows[k]
        xr_core = np.zeros((NODES_PER_CORE, C * D), np.float32)
        valid = rows >= 0
        xr_core[valid] = xr[rows[valid]].reshape(-1, C * D)
        # channel-major x: xt[c, (tile, i, node)]
        xt = xr_core.reshape(TPC, TILE_N, C, D).transpose(2, 0, 3, 1)
        in_maps.append({
            'xt': _bf16(np.ascontiguousarray(xt.reshape(C, TPC * D * TILE_N))),
            'wc': wc_bf,
            'gk': gk_bf,
            'bias': bias,
            'lin': lin_bf,
        })
    return in_maps, device_rows, overflow_idx


# ----------------------------------------------------------------------------
# device kernel
# ----------------------------------------------------------------------------

def build_device(repeat=1, stages=5):
    import concourse.bacc as bacc
    import concourse.mybir as mybir
    from concourse.tile import TileContext

    f32, bf16 = mybir.dt.float32, mybir.dt.bfloat16
    AL = mybir.AluOpType

    nc = bacc.Bacc("TRN2", target_bir_lowering=False, debug=False,
                   num_devices=N_CORES)

    xt_d = nc.dram_tensor('xt', [C, TPC * D * TILE_N], bf16, kind='ExternalInput').ap()
    wc_d = nc.dram_tensor('wc', [C, S * NM55 * 36], bf16, kind='ExternalInput').ap()
    gk_d = nc.dram_tensor('gk', [C, S * 2 * C], bf16, kind='ExternalInput').ap()
    bias_d = nc.dram_tensor('bias', [C, S * 2], f32, kind='ExternalInput').ap()
    lin_d = nc.dram_tensor('lin', [C, 2 * C], bf16, kind='ExternalInput').ap()
    # transposed output: [c, (tile, z, node)]; host un-permutes and casts to f32
    out_d = nc.dram_tensor('out', [C, TPC * 4 * TILE_N], bf16, kind='ExternalOutput').ap()

    TD = D * TILE_N       # 1152: one tile's x block
    TV = NM55 * TILE_N    # 7040: one tile's V block

    with TileContext(nc) as tc:
        with (
            tc.tile_pool(name='const', bufs=1) as constp,
            tc.tile_pool(name='xt', bufs=1) as xtp,
            tc.tile_pool(name='vb', bufs=1) as vbp,
            tc.tile_pool(name='tg', bufs=1) as tgp,
            tc.tile_pool(name='ff', bufs=1) as ffp,
            tc.tile_pool(name='sb', bufs=1) as sbp,
            tc.tile_pool(name='facc', bufs=1) as faccp,
            tc.tile_pool(name='outt', bufs=1) as outp,
            tc.tile_pool(name='ps_misc', bufs=2, space='PSUM') as ps_m,
        ):
            wc_s = constp.tile([C, S * NM55 * 36], bf16)
            nc.sync.dma_start(out=wc_s[:], in_=wc_d[:])
            gk_s = constp.tile([C, S * 2 * C], bf16)
            nc.sync.dma_start(out=gk_s[:], in_=gk_d[:])
            bias_s = constp.tile([C, S * 2], f32)
            nc.sync.dma_start(out=bias_s[:], in_=bias_d[:])
            lin_s = constp.tile([C, 2 * C], bf16)
            nc.sync.dma_start(out=lin_s[:], in_=lin_d[:])

            for rep in range(repeat):
              for (t0, ng) in ((0, 6), (6, 4)):   # tile groups
                xt_t = xtp.tile([C, 6 * TD], bf16)
                nc.sync.dma_start(out=xt_t[:, 0:ng * TD],
                                  in_=xt_d[:, t0 * TD:(t0 + ng) * TD])

                for h in range(t0 // 2, (t0 + ng) // 2):  # pairs (t = 2h, 2h+1)
                    # ---- monomials for both tiles: vb2[c, (t2, m, n)] ----
                    vb2 = vbp.tile([C, 2 * TV], bf16)
                    xb2 = (xt_t[:, (2 * h - t0) * TD:(2 * h - t0 + 2) * TD]
                           .rearrange('p (t inn) -> p t inn', t=2))
                    vb2v = vb2[:, :].rearrange('p (t mn) -> p t mn', t=2)
                    for o in range(D):
                        nj = D - o
                        nc.vector.tensor_mul(
                            vb2v[:, :, OSTART[o] * TILE_N:(OSTART[o] + nj) * TILE_N],
                            xb2[:, :, 0:nj * TILE_N],
                            xb2[:, :, o * TILE_N:(o + nj) * TILE_N])
                    nc.vector.tensor_copy(
                        vb2v[:, :, 45 * TILE_N:54 * TILE_N], xb2)
                    nc.vector.memset(
                        vb2v[:, :, 54 * TILE_N:55 * TILE_N], 1.0)

                    if stages < 2:
                        ot = outp.tile([C, 2 * 4 * TILE_N], bf16, tag='ot')
                        nc.vector.tensor_copy(ot[:, 0:TILE_N], vb2[:, 0:TILE_N])
                        nc.vector.memset(ot[:, TILE_N:], 0.0)
                        nc.sync.dma_start(
                            out=out_d[:, 2 * h * 4 * TILE_N:(2 * h + 2) * 4 * TILE_N],
                            in_=ot[:])
                        continue

                    ot = outp.tile([C, 2 * 4 * TILE_N], bf16, tag='ot')
                    for t2 in range(2):
                        s = 2 * h + t2   # species == tile index
                        # ---- F[c,(z,n,i)] = sum_m V[c,(m,n)] wc[c,(s,m,zi)] ----
                        ff = ffp.tile([C, 4 * TILE_N * D], f32)
                        v4 = (vb2[:, t2 * TV:(t2 + 1) * TV]
                              .rearrange('p (m n) -> p n m', m=NM55).unsqueeze(1))
                        wz = wc_s[:, s * NM55 * 36:(s + 1) * NM55 * 36].rearrange(
                            'p (m zi) -> p zi m', zi=36)
                        ff36 = ff[:, :].rearrange('p (zi n) -> p zi n', zi=36)
                        for zc in range(6):
                            zi0 = zc * 6
                            tg = tgp.tile([C, 6 * TILE_N * NM55], bf16)
                            tg_v = tg[:, :].rearrange(
                                'p (zi n m) -> p zi n m', zi=6, n=TILE_N)
                            nc.vector.tensor_mul(
                                tg_v,
                                v4.broadcast_to([C, 6, TILE_N, NM55]),
                                wz[:, zi0:zi0 + 6, :]
                                .unsqueeze(2).broadcast_to([C, 6, TILE_N, NM55]))
                            nc.vector.tensor_reduce(
                                ff36[:, zi0:zi0 + 6, :], tg_v,
                                axis=mybir.AxisListType.X, op=AL.add)

                        # ---- f[c,(z,n)] = sum_i F[c,(z,n,i)] * x[c,(i,n)] ----
                        gg = sbp.tile([C, 4 * TILE_N * D], bf16, tag='gg')
                        xv = (xt_t[:, (s - t0) * TD:(s - t0 + 1) * TD]
                              .rearrange('p (i n) -> p i n', i=D)
                              .unsqueeze(1).broadcast_to([C, 4, D, TILE_N]))
                        nc.vector.tensor_mul(
                            gg[:, :].rearrange('p (z i n) -> p z i n', z=4, i=D),
                            ff[:, :].rearrange('p (z i n) -> p z i n', z=4, i=D),
                            xv)
                        facc = faccp.tile([C, 4 * TILE_N], bf16)
                        with nc.allow_low_precision(reason='9-elem reduce'):
                            nc.vector.tensor_reduce(
                                facc[:, :].rearrange('p (z n) -> p z n', z=4),
                                gg[:, :].rearrange('p (z i n) -> p z n i', z=4, i=D),
                                axis=mybir.AxisListType.X, op=AL.add)

                        if stages < 5:
                            nc.vector.tensor_copy(
                                ot[:, t2 * 4 * TILE_N:(t2 + 1) * 4 * TILE_N], facc[:])
                            continue

                        # ---- gate matmuls: gate_half^T = gk_half^T @ f0^T ----
                        gps = ps_m.tile([C, 2 * TILE_N], f32, tag='misc')
                        nc.tensor.matmul(gps[:, 0:TILE_N],
                                         gk_s[:, s * 256:s * 256 + 128],
                                         facc[:, 0:TILE_N],
                                         start=True, stop=True)
                        nc.tensor.matmul(gps[:, TILE_N:2 * TILE_N],
                                         gk_s[:, s * 256 + 128:s * 256 + 256],
                                         facc[:, 0:TILE_N],
                                         start=True, stop=True)

                        # ---- fused bias + gating: fg = (gps + bias) * facc ----
                        fg = sbp.tile([C, 4 * TILE_N], bf16, tag='fg')
                        nc.vector.scalar_tensor_tensor(
                            out=fg[:, 0:TILE_N],
                            in0=gps[:, 0:TILE_N],
                            scalar=bias_s[:, 2 * s:2 * s + 1],
                            in1=facc[:, 0:TILE_N],
                            op0=AL.add, op1=AL.mult)
                        nc.vector.scalar_tensor_tensor(
                            out=fg[:, TILE_N:].rearrange('p (zz n) -> p zz n', zz=3),
                            in0=gps[:, TILE_N:2 * TILE_N].unsqueeze(1)
                            .broadcast_to([C, 3, TILE_N]),
                            scalar=bias_s[:, 2 * s + 1:2 * s + 2],
                            in1=facc[:, TILE_N:].rearrange('p (zz n) -> p zz n', zz=3),
                            op0=AL.add, op1=AL.mult)

                        # ---- linear (c-major): out^T [k, (z, n)] ----
                        ops_ = ps_m.tile([C, 4 * TILE_N], f32, tag='misc')
                        nc.tensor.matmul(ops_[:, 0:TILE_N], lin_s[:, 0:C],
                                         fg[:, 0:TILE_N], start=True, stop=True)
                        nc.tensor.matmul(ops_[:, TILE_N:4 * TILE_N], lin_s[:, C:2 * C],
                                         fg[:, TILE_N:4 * TILE_N], start=True, stop=True)
                        nc.vector.tensor_copy(
                            ot[:, t2 * 4 * TILE_N:(t2 + 1) * 4 * TILE_N], ops_[:])
                    nc.sync.dma_start(
                        out=out_d[:, 2 * h * 4 * TILE_N:(2 * h + 2) * 4 * TILE_N],
                        in_=ot[:])

    nc.compile()
    return nc


_NC_CACHE = {}


def _get_device(repeat=1, stages=5):
    key = (repeat, stages)
    if key not in _NC_CACHE:
        _NC_CACHE[key] = build_device(repeat, stages)
    return _NC_CACHE[key]


def kernel(**inputs):
    from concourse.bass_utils import run_bass_kernel_spmd

    in_maps, device_rows, overflow_idx = host_prepare(inputs)
    nc = _get_device(1)
    res = run_bass_kernel_spmd(nc, in_maps, list(range(N_CORES)))

    ntot = np.asarray(inputs['node_species']).shape[0]
    out = np.zeros((ntot, 4 * C), np.float32)
    for k in range(N_CORES):
        rows = device_rows[k]
        valid = rows >= 0
        # device output is [c, (tile, z, node)]; un-permute to [node, 512]
        a = np.asarray(res.results[k]['out'], np.float32).reshape(C, TPC, 4, TILE_N)
        o = np.empty((NODES_PER_CORE, 4 * C), np.float32)
        o[:, :C] = a[:, :, 0, :].transpose(1, 2, 0).reshape(NODES_PER_CORE, C)
        o[:, C:] = a[:, :, 1:4, :].transpose(1, 3, 0, 2).reshape(NODES_PER_CORE, 3 * C)
        out[rows[valid]] = o[valid]
    if len(overflow_idx):
        out[overflow_idx] = _numpy_forward(inputs, overflow_idx)
    return out
